# revision 5
# baseline (speedup 1.0000x reference)
"""CatLayer Trainium2 kernel, b-sharded: each core owns 2 of the 16 batch
rows, so it loads only its x slice (196KB vs 1.57MB) -- both i and j of the
pair grid span all l locally. Transposed (channel-partition) compute,
fp16 output stores.

Math: out[i,j,b,:] = W @ leaky_relu(concat(x[i,b,:], x[j,b,:])) + bias
    y  = leaky_relu(x_own)                # (l, 2, d) -> t' = i*2+b', 192 rows
    A^T[c, t'] = sum_k W1T[k, c] yT[k, t']
    B^T[c, t'] = sum_k W2T[k, c] yT[k, t'] + bias[c]
    out^T[c, (i, j, b')] = A^T[c, i*2+b'] + B^T[c, j*2+b']

The kernel is DMA-bound (shared 360 GB/s DMA device; fp16 out 18.9 MB/core
= 52.4 us, loads now only 1.25 MB = 3.5 us). Loads stream [x | W2ct0+bias+
W1ct0 | W1rest | W2rest]; the whole A/B compute is tiny (8 matmuls of 192
free per c-tile) so the first store chain (x-sem, relu, matmuls, drains,
first small add) limits the start; after that the store stream runs
gap-free to the end.

Engines:
    ACT: the two leaky-relu halves, B^T drains with per-partition bias
         fused (Identity+bias), late A^T drains
    PE : A^T/B^T matmuls (fp16 x fp16 -> fp32 PSUM), warmed up
    DVE: ct0's A^T drain + all output adds (fp16 2x mode)
    SP : all loads + all stores
"""

import numpy as np
from contextlib import ExitStack

import concourse.bacc as bacc
import concourse.mybir as mybir
from concourse import tile
from concourse.bass_utils import run_bass_kernel_spmd

F32 = mybir.dt.float32
F16 = mybir.dt.float16
AF = mybir.ActivationFunctionType

L, Bdim, D = 96, 16, 512
NCORES = 8
BPC = Bdim // NCORES       # 2 batch rows per core
T = L * BPC                # 192 (i, b') rows of this core's y
KT = D // 128              # 4 k-tiles
CT = 4                     # c-tiles of 128 channels
NEG_SLOPE = 0.1
# w_sb/w_in column layout (same spirit as the i-sharded kernel):
#   [0, D) W2ct0 | [D, D+32) bias+pad | [D+32, 2D+32) W1ct0
#   | [2D+32, 5D+32) W1 ct1-3 | [5D+32, 8D+32) W2 ct1-3
BCOL = D
W1CT0 = D + 32
W1REST = W1CT0 + D
W2REST = W1REST + 3 * D
WCOLS = W2REST + 3 * D

DEF_UNITS = (
    (5, 6, 6, 8, 8, 8, 16, 16, 16, 7),
    (16, 16, 16, 16, 16, 16),
    (16, 16, 16, 16, 16, 16),
    (16, 16, 16, 16, 16, 16),
)


def build_nc(n_warm=26, units=DEF_UNITS, n_psum_units=0):
    """units[ct] = tuple of i-range sizes for that c-tile's add/store units
    (each >= 2 to keep DRAM runs >= 512B); sizes must sum to L."""
    nc = bacc.Bacc("TRN2", target_bir_lowering=False, debug=False)

    xT = nc.dram_tensor("xT", (128, KT * T), F16, kind="ExternalInput")
    w_in = nc.dram_tensor("w_in", (128, WCOLS), F16, kind="ExternalInput")
    # out[c, i*192 + j*2 + b'] = out^T[c, i, j, b']
    out = nc.dram_tensor("out", (D, L * T), F16, kind="ExternalOutput")

    with tile.TileContext(nc) as tc, ExitStack() as ctx:
        persist = ctx.enter_context(tc.tile_pool(name="persist", bufs=1))
        psum = ctx.enter_context(tc.tile_pool(name="psum", bufs=5, space="PSUM"))
        outp = ctx.enter_context(tc.tile_pool(name="outp", bufs=1))

        ones_sb = persist.tile([1, 128], F16, tag="ones", name="ones_sb")
        nc.vector.memset(ones_sb[:], 1.0)

        w_sb = persist.tile([128, WCOLS], F16, tag="w", name="w_sb")
        biasT_sb = w_sb[:, BCOL : BCOL + CT]
        # fp32 copy of the bias for DVE tensor_scalar_add (requires f32 scalar)
        bias32 = persist.tile([128, CT], F32, tag="b32", name="bias32")

        def w1s(k, ct):
            c0 = (W1CT0 if ct == 0 else W1REST + (ct - 1) * D) + k * 128
            return w_sb[:, c0 : c0 + 128]

        def w2s(k, ct):
            c0 = (0 if ct == 0 else W2REST + (ct - 1) * D) + k * 128
            return w_sb[:, c0 : c0 + 128]

        warm_ps = psum.tile([128, 128], F32, tag="warm", bufs=1, name="warm_ps")
        for _ in range(n_warm):
            nc.tensor.matmul(
                warm_ps[:], ones_sb[:1, :], ones_sb[:1, :], start=True, stop=True
            )

        # ---- loads: x first (gates the relu), then the W block the first
        # matmuls need, then the W tails.
        x_st = persist.tile([128, KT * T], F16, tag="x_st", name="x_st")
        nc.sync.dma_start(x_st[:], xT.ap())
        nc.sync.dma_start(w_sb[:, :W1REST], w_in[:, :W1REST])
        nc.sync.dma_start(w_sb[:, W1REST:W2REST], w_in[:, W1REST:W2REST])
        nc.sync.dma_start(w_sb[:, W2REST:], w_in[:, W2REST:])

        yT = persist.tile([128, KT * T], F16, tag="yT", name="yT")
        # relu halves split ACROSS ENGINES (k01 on ACT, k23 on DVE as
        # (x*0.1) max x) so both finish ~0.5us after the x DMA sem and the
        # scheduler never mispredicts the ACT queue (act-table load) into
        # head-blocking ct0's matmuls behind W-tail-gated ones.
        nc.scalar.activation(
            yT[:, : 2 * T], x_st[:, : 2 * T], AF.Prelu, alpha=NEG_SLOPE
        )
        nc.vector.tensor_copy(bias32[:], biasT_sb)
        nc.vector.scalar_tensor_tensor(
            yT[:, 2 * T :],
            x_st[:, 2 * T :],
            NEG_SLOPE,
            x_st[:, 2 * T :],
            mybir.AluOpType.mult,
            mybir.AluOpType.max,
        )

        # A^T and B^T share one PSUM bank per c-tile: cols [0,192) = A,
        # [192, 384) = B.
        ab_sb = persist.tile([128, CT * 2 * T], F16, tag="ab", name="ab_sb")

        def a_sl_of(ct):
            return ab_sb[:, ct * 2 * T : ct * 2 * T + T]

        def b_sl_of(ct):
            return ab_sb[:, ct * 2 * T + T : (ct + 1) * 2 * T]

        def emit_ab(ct):
            # B first: its drain is the first gate of the ct's add stream.
            # SEPARATE psum tiles for A and B -- a shared tile has no
            # sub-range dep tracking, so B's drain would serialize behind
            # A's matmuls.
            bps = psum.tile([128, T], F32, tag="bps", bufs=3, name=f"bps_{ct}")
            aps = psum.tile([128, T], F32, tag="aps", bufs=3, name=f"aps_{ct}")
            for k in range(KT):
                nc.tensor.matmul(
                    bps[:], w2s(k, ct), yT[:, k * T : (k + 1) * T],
                    start=(k == 0), stop=(k == KT - 1),
                )
            for k in range(KT):
                nc.tensor.matmul(
                    aps[:], w1s(k, ct), yT[:, k * T : (k + 1) * T],
                    start=(k == 0), stop=(k == KT - 1),
                )
            # ct0: both drains on DVE, the same queue as the adds -- no
            # cross-engine sem hop on the first-store critical path; the
            # bias rides tensor_scalar_add. ct1-3: drains on ACT (idle).
            if ct == 0:
                nc.vector.tensor_scalar_add(
                    b_sl_of(ct), bps[:], bias32[:, ct : ct + 1]
                )
                nc.vector.tensor_copy(a_sl_of(ct), aps[:])
            else:
                nc.scalar.add(b_sl_of(ct), bps[:], biasT_sb[:, ct : ct + 1])
                nc.scalar.activation(a_sl_of(ct), aps[:], AF.Copy)
            return None

        def emit_unit(ct, i0, ni, ps=None):
            ot = outp.tile([128, ni * T], F16, tag="ot", bufs=8,
                           name=f"ot_{ct}_{i0}")
            if ps is not None:
                # first unit(s): add straight from the A/B PSUM halves,
                # skipping both drains on the first-store critical path
                a_src, b_src = ps[:, :T], ps[:, T:]
            else:
                a_src, b_src = a_sl_of(ct), b_sl_of(ct)
            a_sl = (
                a_src[:, i0 * BPC : (i0 + ni) * BPC]
                .rearrange("p (i b) -> p i b", b=BPC)
                .unsqueeze(2)
                .broadcast_to((128, ni, L, BPC))
            )
            b_sl = (
                b_src
                .rearrange("p (j b) -> p j b", b=BPC)
                .unsqueeze(1)
                .broadcast_to((128, ni, L, BPC))
            )
            nc.vector.tensor_add(
                ot[:].rearrange("p (i j b) -> p i j b", j=L, b=BPC), b_sl, a_sl
            )
            nc.sync.dma_start(
                out.ap()[ct * 128 : (ct + 1) * 128, i0 * T : (i0 + ni) * T],
                ot[:],
            )

        for ct in range(CT):
            ps = emit_ab(ct)
            assert ps is None
            i0 = 0
            for u, ni in enumerate(units[ct]):
                emit_unit(ct, i0, ni,
                          ps if (ct == 0 and u < n_psum_units) else None)
                i0 += ni
            assert i0 == L

    nc.compile()
    return nc


def _pack_kt(arr_t, nfree):
    """(D, nfree) k-major -> (128, KT*nfree) partition-packed SBUF layout."""
    return np.ascontiguousarray(
        arr_t.reshape(KT, 128, nfree).transpose(1, 0, 2).reshape(128, KT * nfree)
    )


def make_in_maps(x, W, bias):
    x = np.asarray(x, np.float32)
    W = np.asarray(W, np.float32)
    bias = np.asarray(bias, np.float32)
    wt = np.ascontiguousarray(W.T.astype(np.float16))       # (2d, d) = (k_full, c)
    w1 = wt[:D].reshape(KT, 128, CT, 128).transpose(1, 2, 0, 3).reshape(128, KT * D)
    w2 = wt[D:].reshape(KT, 128, CT, 128).transpose(1, 2, 0, 3).reshape(128, KT * D)
    bias_blk = np.zeros((128, 32), np.float16)
    bias_blk[:, :CT] = bias.reshape(CT, 128).T.astype(np.float16)
    w_all = np.ascontiguousarray(
        np.concatenate(
            [w2[:, :D], bias_blk, w1, w2[:, D:]], axis=1, dtype=np.float16
        )
    )
    maps = []
    for r in range(NCORES):
        xs = x[:, r * BPC : (r + 1) * BPC, :].reshape(T, D).astype(np.float16)
        xTr = _pack_kt(np.ascontiguousarray(xs.T), T)
        maps.append({"xT": xTr, "w_in": w_all})
    return maps


_NC_CACHE = {}


def get_nc(repeats=1, **kw):
    key = (repeats, tuple(sorted(kw.items())))
    if key not in _NC_CACHE:
        _NC_CACHE[key] = build_nc(**kw)
    return _NC_CACHE[key]


def kernel(x, W, bias, **kw):
    nc = get_nc(1, **kw)
    maps = make_in_maps(x, W, bias)
    res = run_bass_kernel_spmd(nc, maps, list(range(NCORES)))
    outs = []
    for r in range(NCORES):
        o = res.results[r]["out"].reshape(D, L, L, BPC)
        outs.append(o.transpose(1, 2, 3, 0))    # (i, j, b', c)
    full = np.concatenate(outs, axis=2).astype(np.float32)
    return np.ascontiguousarray(full.reshape(L * L, Bdim, D))
